# revision 6
# baseline (speedup 1.0000x reference)
"""MI-LSTM (attention LSTM) + LSTM + linear head for Trainium2, 8-core batch-parallel.

Model (per timestep, per batch row b):
  gm = y@W_main + h@U_main + b_main -> i,f,o,cm gates
  ga[k] = x_k@W_aux[k] + h@U_aux[k] + b_aux[k] -> i_k (sigmoid), c_k (tanh)
  candidates l = [i*cm, i_k*c_k] (9, H)
  u_k = tanh(l_k . (W_att @ c) + b_att); a = softmax(u); L = sum a_k l_k
  c' = f*c + L; h' = o*tanh(c')
Then a standard LSTM over the h-sequence, then relu + linear to scalar.

Mapping: batch sharded 8 ways (256 rows/core = 2 partition tiles of 128),
batch-major (batch on partitions). Per step ONE stacked lhsT tile XYHT
(rows 0:45 x/y feature-major loaded pre-transposed from DRAM, row 45 ones
for biases, rows 64:128 h1^T) feeds 3x512-col matmuls per tile covering
phase-1 gates AND the phase-2 input projection; the phase-2 LSTM is
interleaved into the phase-1 loop one step behind, so its latency hides
in phase-1's engine bubbles.

All sigmoids are computed as tanh(x/2) (the 1/2 folded into the constant
weights), so the only ACT functions used are tanh/exp/identity -- all in
one ACT table set; softmax uses exp directly. Scale bookkeeping:
  l' = (tanh(gi)+1)*tanh(gc) = 2*l          (candidates, stt)
  v' = c @ (0.5*W_att^T)                    (0.5 folded in weights)
  u  = l'.v' (exact), ua = tanh(u + b_att)
  r' = exp(ua + ln2) = 2*exp(ua);  s' = sum r' = 2s;  rs = 1/s' = 0.5/s
  aw = (l'*0.5)*r' = l*r' = 2*l*exp(ua)     (stt)
  L  = (sum_k aw) * rs                      (exact softmax-weighted sum)
"""

import os
import numpy as np
import ml_dtypes

import concourse.bacc as bacc
import concourse.bass as bass
import concourse.mybir as mybir
from concourse.tile import TileContext
from concourse.bass_utils import run_bass_kernel_spmd

F32 = mybir.dt.float32
BF16 = mybir.dt.bfloat16
ALU = mybir.AluOpType
ACTF = mybir.ActivationFunctionType
AX = mybir.AxisListType

S, B, F, H, K = 256, 2048, 5, 64, 8
NC = 8
BL = B // NC          # 256 batch rows per core
NT = BL // 128        # 2 partition tiles
NCAND = K + 1         # 9 candidates
LN2 = 0.6931471805599453

LAST_RESULTS = {}


def _build(n_steps: int, b_att: float):
    nc = bacc.Bacc(None, target_bir_lowering=False)

    xin = nc.dram_tensor("xin", [n_steps, 45, BL], BF16, kind="ExternalInput")
    wall = nc.dram_tensor("wall", [128, 1536], BF16, kind="ExternalInput")
    wcb = nc.dram_tensor("wcb", [H, 4 * H], BF16, kind="ExternalInput")
    wa05 = nc.dram_tensor("wa05", [H, H], BF16, kind="ExternalInput")
    linw = nc.dram_tensor("linw", [128, H], BF16, kind="ExternalInput")
    idf32 = nc.dram_tensor("idf32", [128, 128], F32, kind="ExternalInput")
    onesrow = nc.dram_tensor("onesrow", [1, BL], BF16, kind="ExternalInput")
    out = nc.dram_tensor("out", [n_steps, BL, 1], F32, kind="ExternalOutput")

    with TileContext(nc) as tc:
        with (
            tc.tile_pool(name="state", bufs=1) as st,
            tc.tile_pool(name="wts", bufs=1) as wp,
            tc.tile_pool(name="work", bufs=2) as wk,
            tc.tile_pool(name="gpsum", bufs=1, space="PSUM") as gp,
            tc.tile_pool(name="mpsum", bufs=1, space="PSUM") as mp,
            tc.tile_pool(name="vpsum", bufs=1, space="PSUM") as vp,
        ):
            # ---- persistent weights in SBUF ----
            W = wp.tile([128, 1536], BF16, tag="wall")
            WCB = wp.tile([H, 4 * H], BF16, tag="wcb")
            WA = wp.tile([H, H], BF16, tag="wa05")
            LW = wp.tile([128, H], BF16, tag="linw")
            IDF = wp.tile([128, 128], F32, tag="idf32")
            for t_, d_ in ((W, wall), (WCB, wcb), (WA, wa05), (LW, linw),
                           (IDF, idf32)):
                nc.sync.dma_start(t_[:], d_[:])

            BATT = wp.tile([128, 1], F32, tag="batt")
            BLN2 = wp.tile([128, 1], F32, tag="bln2")
            nc.vector.memset(BATT[:], b_att)
            nc.vector.memset(BLN2[:], LN2)

            XYHT = st.tile([128, BL], BF16, tag="xyht")   # stacked lhsT
            CT = st.tile([H, BL], BF16, tag="ct")         # c1^T for v matmul
            H2T = st.tile([H, BL], BF16, tag="h2t")       # h2^T for p2 matmul
            HC1 = st.tile([128, NT * 128], F32, tag="hc1")  # [h1|c1] per tile
            HC2 = st.tile([128, NT * 128], F32, tag="hc2")  # [h2|c2] per tile
            OACC = st.tile([128, NT * n_steps], F32, tag="oacc")

            nc.vector.memset(XYHT[32:64, :], 0.0)
            nc.vector.memset(XYHT[64:128, :], 0.0)
            nc.sync.dma_start(XYHT[45:46, :], onesrow[:])
            nc.vector.memset(HC1[:], 0.0)
            nc.vector.memset(HC2[:], 0.0)

            hc1v = HC1[:].rearrange("p (t x) -> p t x", t=NT)
            hc2v = HC2[:].rearrange("p (t x) -> p t x", t=NT)

            for t in range(n_steps + 1):
                p1 = t < n_steps
                p2 = t >= 1

                if p1:
                    nc.sync.dma_start(XYHT[0:45, :], xin[t])

                # transposes: misc cols 0:256 = [h1|c1], 256:512 = h2^T
                misc = mp.tile([128, 512], F32, tag="misc")
                for tau in range(NT):
                    nc.tensor.transpose(
                        misc[0:128, tau * 128:(tau + 1) * 128],
                        HC1[:, tau * 128:(tau + 1) * 128], IDF[:])
                if p2:
                    for tau in range(NT):
                        nc.tensor.transpose(
                            misc[0:64, 256 + tau * 128:256 + (tau + 1) * 128],
                            HC2[:, tau * 128:tau * 128 + 64], IDF[:])

                nc.vector.tensor_copy(XYHT[64:128, :], misc[0:64, 0:256])
                if p1:
                    nc.vector.tensor_copy(CT[:], misc[64:128, 0:256])
                if p2:
                    nc.vector.tensor_copy(H2T[:], misc[0:64, 256:512])

                # matmuls; gps per tile: [0:1152 l-gates | 1152:1280 f,o |
                #                         1280:1536 phase-2 gates]
                gps = gp.tile([128, NT * 1536], F32, tag="gates")
                if p1:
                    vps = vp.tile([128, 128], F32, tag="v")
                    for tau in range(NT):
                        nc.tensor.matmul(vps[:, tau * 64:(tau + 1) * 64],
                                         CT[:, tau * 128:(tau + 1) * 128],
                                         WA[:], start=True, stop=True)
                for tau in range(NT):
                    b0 = tau * 1536
                    lhsT = XYHT[:, tau * 128:(tau + 1) * 128]
                    if p1:
                        nc.tensor.matmul(gps[:, b0:b0 + 512], lhsT,
                                         W[:, 0:512], start=True, stop=True)
                        nc.tensor.matmul(gps[:, b0 + 512:b0 + 1024], lhsT,
                                         W[:, 512:1024], start=True, stop=True)
                        nc.tensor.matmul(gps[:, b0 + 1024:b0 + 1280], lhsT,
                                         W[:, 1024:1280], start=True, stop=True)
                    if p2:
                        nc.tensor.matmul(gps[:, b0 + 1280:b0 + 1536], lhsT,
                                         W[:, 1280:1536], start=True, stop=False)
                        nc.tensor.matmul(gps[:, b0 + 1280:b0 + 1536],
                                         H2T[:, tau * 128:(tau + 1) * 128],
                                         WCB[:], start=False, stop=True)

                glv = gps[:].rearrange("p (t c) -> p t c", t=NT)

                if p1:
                    # all gate nonlinearities are tanh (sigmoid = scaled tanh)
                    LT = wk.tile([128, NT * 1152], BF16, tag="lt")
                    ltv = LT[:].rearrange("p (t c) -> p t c", t=NT)
                    nc.scalar.activation(ltv, glv[:, :, 0:1152], ACTF.Tanh)
                    FOt = wk.tile([128, NT * 128], BF16, tag="fot")
                    nc.scalar.activation(
                        FOt[:].rearrange("p (t c) -> p t c", t=NT),
                        glv[:, :, 1152:1280], ACTF.Tanh)
                    FOs = wk.tile([128, NT * 128], BF16, tag="fos")
                    fosv = FOs[:].rearrange("p (t c) -> p t c", t=NT)
                    nc.vector.tensor_scalar(FOs[:], FOt[:], 1.0, 0.5,
                                            ALU.add, ALU.mult)

                    # candidates l' = (tanh_i + 1) * tanh_c  (= 2*l)
                    lc = wk.tile([128, NT * 576], BF16, tag="lc")
                    nc.vector.scalar_tensor_tensor(
                        lc[:].rearrange("p (t c) -> p t c", t=NT),
                        ltv[:, :, 0:576], 1.0, ltv[:, :, 576:1152],
                        ALU.add, ALU.mult)

                    # attention: u = l'.v'
                    VB = wk.tile([128, 128], BF16, tag="vb")
                    nc.vector.tensor_copy(VB[:], vps[:])
                    z = wk.tile([128, NT * 576], BF16, tag="z")
                    nc.vector.tensor_tensor(
                        z[:].rearrange("p (t k h) -> p t k h", k=NCAND, h=H),
                        lc[:].rearrange("p (t k h) -> p t k h", k=NCAND, h=H),
                        (VB[:].rearrange("p (t h) -> p t h", t=NT)
                         .unsqueeze(2).broadcast_to((128, NT, NCAND, H))),
                        ALU.mult)
                    u_t = wk.tile([128, NT * NCAND], F32, tag="u")
                    nc.vector.tensor_reduce(
                        u_t[:],
                        z[:].rearrange("p (t k h) -> p t k h", k=NCAND, h=H),
                        AX.X, ALU.add)
                    ua = wk.tile([128, NT * NCAND], F32, tag="ua")
                    nc.scalar.activation(ua[:], u_t[:], ACTF.Tanh, bias=BATT[:])
                    r_t = wk.tile([128, NT * NCAND], BF16, tag="r")
                    nc.scalar.activation(r_t[:], ua[:], ACTF.Exp, bias=BLN2[:])
                    s_t = wk.tile([128, NT], F32, tag="s")
                    nc.vector.tensor_reduce(
                        s_t[:],
                        r_t[:].rearrange("p (t k) -> p t k", t=NT),
                        AX.X, ALU.add)
                    rs = wk.tile([128, NT], F32, tag="rs")
                    nc.vector.reciprocal_approx_fast(rs[:], s_t[:])

                    aw = wk.tile([128, NT * 576], BF16, tag="aw")
                    nc.vector.scalar_tensor_tensor(
                        aw[:].rearrange("p (t k h) -> p t k h", k=NCAND, h=H),
                        lc[:].rearrange("p (t k h) -> p t k h", k=NCAND, h=H),
                        0.5,
                        (r_t[:].rearrange("p (t k) -> p t k", t=NT)
                         .unsqueeze(3).broadcast_to((128, NT, NCAND, H))),
                        ALU.mult, ALU.mult)
                    Lp = wk.tile([128, NT * H], F32, tag="L")
                    nc.vector.tensor_reduce(
                        Lp[:],
                        aw[:].rearrange("p (t k h) -> p t h k", k=NCAND, h=H),
                        AX.X, ALU.add)

                    # c1' = f*c1 + rs*Lp ; h1' = o*tanh(c1')
                    fc = wk.tile([128, NT * H], F32, tag="fc")
                    nc.vector.tensor_tensor(
                        fc[:].rearrange("p (t h) -> p t h", t=NT),
                        fosv[:, :, 0:64], hc1v[:, :, 64:128], ALU.mult)
                    for tau in range(NT):
                        nc.vector.scalar_tensor_tensor(
                            HC1[:, tau * 128 + 64:tau * 128 + 128],
                            Lp[:, tau * H:(tau + 1) * H], rs[:, tau:tau + 1],
                            fc[:, tau * H:(tau + 1) * H], ALU.mult, ALU.add)
                    TC1 = wk.tile([128, NT * H], BF16, tag="tc1")
                    nc.scalar.activation(
                        TC1[:].rearrange("p (t h) -> p t h", t=NT),
                        hc1v[:, :, 64:128], ACTF.Tanh)
                    nc.vector.tensor_tensor(
                        hc1v[:, :, 0:64], fosv[:, :, 64:128],
                        TC1[:].rearrange("p (t h) -> p t h", t=NT), ALU.mult)

                if p2:
                    # phase-2 LSTM step t-1; gates [i f o g] all tanh'd
                    G2 = wk.tile([128, NT * 256], BF16, tag="g2")
                    g2v = G2[:].rearrange("p (t c) -> p t c", t=NT)
                    nc.scalar.activation(g2v, glv[:, :, 1280:1536], ACTF.Tanh)
                    ig2 = wk.tile([128, NT * H], F32, tag="ig2")
                    nc.vector.scalar_tensor_tensor(
                        ig2[:].rearrange("p (t h) -> p t h", t=NT),
                        g2v[:, :, 0:64], 1.0, g2v[:, :, 192:256],
                        ALU.add, ALU.mult)
                    fo2 = wk.tile([128, NT * 128], BF16, tag="fo2")
                    fo2v = fo2[:].rearrange("p (t c) -> p t c", t=NT)
                    nc.vector.tensor_scalar(
                        fo2v, g2v[:, :, 64:192], 1.0, 0.5, ALU.add, ALU.mult)
                    fc2 = wk.tile([128, NT * H], F32, tag="fc2")
                    nc.vector.tensor_tensor(
                        fc2[:].rearrange("p (t h) -> p t h", t=NT),
                        fo2v[:, :, 0:64], hc2v[:, :, 64:128], ALU.mult)
                    nc.vector.scalar_tensor_tensor(
                        hc2v[:, :, 64:128],
                        ig2[:].rearrange("p (t h) -> p t h", t=NT), 0.5,
                        fc2[:].rearrange("p (t h) -> p t h", t=NT),
                        ALU.mult, ALU.add)
                    TC2 = wk.tile([128, NT * H], BF16, tag="tc2")
                    nc.scalar.activation(
                        TC2[:].rearrange("p (t h) -> p t h", t=NT),
                        hc2v[:, :, 64:128], ACTF.Tanh)
                    nc.vector.tensor_tensor(
                        hc2v[:, :, 0:64], fo2v[:, :, 64:128],
                        TC2[:].rearrange("p (t h) -> p t h", t=NT), ALU.mult)
                    # out_{t-1} = sum_h relu(h2) * lin_w
                    zz = wk.tile([128, H], F32, tag="zz")
                    for tau in range(NT):
                        nc.vector.scalar_tensor_tensor(
                            zz[:], HC2[:, tau * 128:tau * 128 + 64], 0.0,
                            LW[:], ALU.max, ALU.mult,
                            accum_out=OACC[:, tau * n_steps + t - 1:
                                           tau * n_steps + t])

            ov = out.rearrange("s (tau p) o -> tau p (s o)", p=128)
            for tau in range(NT):
                nc.sync.dma_start(
                    ov[tau], OACC[:, tau * n_steps:(tau + 1) * n_steps])

    nc.finalize()
    return nc


def _prep_weights(inp):
    f32 = np.float32
    W_main, U_main, b_main = (np.asarray(inp["W_main"], f32),
                              np.asarray(inp["U_main"], f32),
                              np.asarray(inp["b_main"], f32))
    W_aux, U_aux, b_aux = (np.asarray(inp["W_aux"], f32),
                           np.asarray(inp["U_aux"], f32),
                           np.asarray(inp["b_aux"], f32))
    # column layout: [i_k(512) | i_main(64) | c_k(512) | cm(64) | f | o | p2(256)]
    # sigmoid-typed columns (i_k, i_main, f, o; p2 i,f,o) are scaled by 0.5.
    wall = np.zeros((128, 1536), f32)
    for k in range(K):
        c = 64 * k
        wall[5 + 5 * k:10 + 5 * k, c:c + 64] = 0.5 * W_aux[k, :, 0:64]
        wall[64:128, c:c + 64] = 0.5 * U_aux[k, :, 0:64]
        wall[45, c:c + 64] = 0.5 * b_aux[k, 0:64]
        wall[5 + 5 * k:10 + 5 * k, 576 + c:576 + c + 64] = W_aux[k, :, 64:128]
        wall[64:128, 576 + c:576 + c + 64] = U_aux[k, :, 64:128]
        wall[45, 576 + c:576 + c + 64] = b_aux[k, 64:128]
    # main gates: W_main cols [i(0:64) f(64:128) o(128:192) cm(192:256)]
    for dst, src, sc in ((512, 0, 0.5), (1088, 192, 1.0),
                         (1152, 64, 0.5), (1216, 128, 0.5)):
        wall[0:5, dst:dst + 64] = sc * W_main[:, src:src + 64]
        wall[64:128, dst:dst + 64] = sc * U_main[:, src:src + 64]
        wall[45, dst:dst + 64] = sc * b_main[src:src + 64]
    # phase-2 input projection: cols [i f o | g], i/f/o scaled 0.5
    perm = np.concatenate([np.arange(0, 128), np.arange(192, 256),
                           np.arange(128, 192)])
    p2scale = np.concatenate([np.full(192, 0.5, f32), np.ones(64, f32)])
    wall[64:128, 1280:1536] = np.asarray(inp["W_ih"], f32).T[:, perm] * p2scale
    wall[45, 1280:1536] = ((np.asarray(inp["b_ih"], f32)
                            + np.asarray(inp["b_hh"], f32))[perm] * p2scale)
    wcb = np.asarray(inp["W_hh"], f32).T[:, perm] * p2scale

    wa05 = 0.5 * np.asarray(inp["W_att"], f32).T
    linw = np.broadcast_to(np.asarray(inp["lin_W"], f32), (128, H)).copy()

    bf = ml_dtypes.bfloat16
    return dict(
        wall=wall.astype(bf), wcb=wcb.astype(bf), wa05=wa05.astype(bf),
        linw=linw.astype(bf), onesrow=np.ones((1, BL), bf),
        idf32=np.eye(128, dtype=np.float32),
    )


def kernel(**inputs) -> np.ndarray:
    n_steps = int(os.environ.get("KERNEL_STEPS", S))
    names = ["Y"] + ["x%d" % i for i in range(1, 9)]
    # (n_steps, 45, B) bf16, feature-major: rows [y(5), x1(5), ..., x8(5)]
    big = np.stack([np.asarray(inputs[n], np.float32)[:n_steps] for n in names],
                   axis=1)  # (n_steps, 9, B, F)
    xall = np.ascontiguousarray(
        big.transpose(0, 1, 3, 2).reshape(n_steps, 45, B)
    ).astype(ml_dtypes.bfloat16)
    wmaps = _prep_weights(inputs)
    b_att = float(np.asarray(inputs["b_att"]).reshape(-1)[0])
    lin_b = float(np.asarray(inputs["lin_b"]).reshape(-1)[0])

    nc = _build(n_steps, b_att)
    in_maps = []
    for c in range(NC):
        m = dict(wmaps)
        m["xin"] = np.ascontiguousarray(xall[:, :, c * BL:(c + 1) * BL])
        in_maps.append(m)

    trace = bool(int(os.environ.get("KERNEL_TRACE", "0")))
    res = run_bass_kernel_spmd(nc, in_maps, core_ids=list(range(NC)),
                               trace=trace)
    LAST_RESULTS["exec_time_ns"] = res.exec_time_ns
    LAST_RESULTS["trace"] = res.instructions_and_trace

    outs = [r["out"] for r in res.results]  # each (n_steps, BL, 1)
    full = np.concatenate(outs, axis=1) + lin_b
    return full.astype(np.float32)
